# revision 2
# baseline (speedup 1.0000x reference)
"""Trainium2 Bass kernel for the binarized MLP (BNN) problem.

Network (eval mode):
  h1 = sign(bn1(x @ sign(w1).T + b1))        x: [8192, 784]
  h2 = sign(bn2(h1 @ sign(w2).T + b2))       hidden: 6144
  h3 = sign(bn3(h2 @ sign(w3).T + b3))
  out = log_softmax(h3 @ w4.T + b4)          out: [8192, 10]
(clip(-1,1) before sign does not change sign, so it is dropped.)

Strategy:
  * Data-parallel over the batch: 8 cores x 1024 rows, no collectives.
  * All activations live transposed in SBUF as hT[H, B] so each layer's
    output feeds the next layer's matmul rhs directly (zero transposes).
  * BN + bias + clip + binarize folds to sign(h*s + c) with
    s = g*rsqrt(v+eps), c = (b - m)*s + be  -> one scalar-engine
    activation (Sign) per psum tile with per-partition scale/bias.
  * Layer 1 (real-valued x, contraction 784->pad 896): x is split into
    3 bf16 planes (hi/mid/lo) so the bf16 matmuls reproduce fp32
    precision (residual error 2^-27); weights are exact +-1 in bf16.
  * Layers 2/3 (+-1 x +-1, contraction 6144): fp8e4m3 with DoubleRow
    perf mode - products and fp32 PSUM accumulation are exact.
  * Layer 4: w4 split hi/lo bf16, h3 (+-1, exact) converted fp8->bf16
    per tile; bias b4 enters via an extra contraction row. log_softmax
    on-device (reduce_max, Exp with accumulate, Ln, subtract).
"""

import numpy as np
import ml_dtypes

H = 6144
B_TOTAL = 8192
N_CORES = 8
B = B_TOTAL // N_CORES  # 1024 rows per core
K1 = 784
K1P = 896  # 7 * 128
EPS = 1e-5
P = 128
M_TILES = H // P  # 48
NB = B // 512  # psum-width chunks per core
BCH = B // P  # 8 output row-chunks per core

_BF16 = ml_dtypes.bfloat16
_FP8 = ml_dtypes.float8_e4m3


def _binarize(w):
    return np.where(w >= 0, np.float32(1.0), np.float32(-1.0))


def _pack_weight(wb, kpad, dtype):
    """[Hout, K] +-1 matrix -> [Hout/128, 128, kpad/128, 128] tiles where
    pack[m, p, ko, j] = wb[m*128 + j, ko*128 + p] (lhsT layout)."""
    hout, k = wb.shape
    if k < kpad:
        wb = np.concatenate([wb, np.zeros((hout, kpad - k), np.float32)], axis=1)
    return np.ascontiguousarray(
        wb.reshape(hout // P, P, kpad // P, P).transpose(0, 3, 2, 1)
    ).astype(dtype)


def _pack_rhs(xc):
    """[B, K] -> [128, K/128, B] with pack[p, ko, b] = xc[b, ko*128+p]."""
    b, k = xc.shape
    return np.ascontiguousarray(xc.T.reshape(k // P, P, b).transpose(1, 0, 2))


def build_nc():
    """Build the (single-program, run-on-8-cores) Bass kernel."""
    import concourse.bass as bass
    import concourse.tile as tile
    import concourse.mybir as mybir
    from concourse import bacc

    af = mybir.ActivationFunctionType
    f32 = mybir.dt.float32
    bf16 = mybir.dt.bfloat16
    f8 = mybir.dt.float8e4

    nc = bacc.Bacc(
        "TRN2",
        target_bir_lowering=False,
        debug=False,
        enable_asserts=False,
        num_devices=N_CORES,
    )

    t = {}
    for nm in ("xhi", "xmid", "xlo"):
        t[nm] = nc.dram_tensor(nm, [P, K1P // P, B], bf16, kind="ExternalInput").ap()
    t["w1p"] = nc.dram_tensor(
        "w1p", [M_TILES, P, K1P // P, P], bf16, kind="ExternalInput"
    ).ap()
    for nm in ("w2p", "w3p"):
        t[nm] = nc.dram_tensor(
            nm, [M_TILES, P, M_TILES, P], f8, kind="ExternalInput"
        ).ap()
    t["w4hl"] = nc.dram_tensor(
        "w4hl", [P, 2, M_TILES + 1, 10], bf16, kind="ExternalInput"
    ).ap()
    for i in (1, 2, 3):
        t[f"s{i}"] = nc.dram_tensor(f"s{i}", [P, M_TILES], f32, kind="ExternalInput").ap()
        t[f"c{i}"] = nc.dram_tensor(f"c{i}", [P, M_TILES], f32, kind="ExternalInput").ap()
    t["out"] = nc.dram_tensor("out", [B, 10], f32, kind="ExternalOutput").ap()

    from contextlib import ExitStack

    with tile.TileContext(nc) as tc, ExitStack() as ctx:
        consts = ctx.enter_context(tc.tile_pool(name="consts", bufs=1))
        xpool = ctx.enter_context(tc.tile_pool(name="x", bufs=1))
        hpool = ctx.enter_context(tc.tile_pool(name="h", bufs=2))
        w1pool = ctx.enter_context(tc.tile_pool(name="w1", bufs=2))
        wpool = ctx.enter_context(tc.tile_pool(name="w", bufs=3))
        pspool = ctx.enter_context(tc.tile_pool(name="ps", bufs=2, space="PSUM"))
        ps4pool = ctx.enter_context(tc.tile_pool(name="ps4", bufs=2, space="PSUM"))
        l4pool = ctx.enter_context(tc.tile_pool(name="l4", bufs=4))
        small = ctx.enter_context(tc.tile_pool(name="small", bufs=4))

        # ---- one-time loads ----
        bn = []
        for i in (1, 2, 3):
            s_t = consts.tile([P, M_TILES], f32, tag=f"s{i}")
            nc.sync.dma_start(s_t[:], t[f"s{i}"][:])
            c_t = consts.tile([P, M_TILES], f32, tag=f"c{i}")
            nc.sync.dma_start(c_t[:], t[f"c{i}"][:])
            bn.append((s_t, c_t))
        w4t = consts.tile([P, 2, M_TILES + 1, 10], bf16, tag="w4")
        nc.sync.dma_start(w4t[:], t["w4hl"][:])
        xs = []
        for nm in ("xhi", "xmid", "xlo"):
            xt = xpool.tile([P, K1P // P, B], bf16, tag=nm)
            nc.sync.dma_start(xt[:], t[nm][:])
            xs.append(xt)
        ones_ext = consts.tile([P, P], bf16, tag="ones")
        nc.vector.memset(ones_ext[:], 0.0)
        nc.vector.memset(ones_ext[0:1, :], 1.0)

        # ---- layer 1: 3-way bf16 split of x, K = 896 ----
        s_t, c_t = bn[0]
        h1 = hpool.tile([P, M_TILES, B], f8, tag="h")
        for m in range(M_TILES):
            wt = w1pool.tile([P, K1P // P, P], bf16, tag="w1")
            nc.sync.dma_start(wt[:], t["w1p"][m])
            for n in range(NB):
                ps = pspool.tile([P, 512], f32, tag="ps")
                for k in range(K1P // P):
                    for si, xt in enumerate(xs):
                        nc.tensor.matmul(
                            ps[:],
                            wt[:, k, :],
                            xt[:, k, n * 512 : (n + 1) * 512],
                            start=(k == 0 and si == 0),
                            stop=(k == K1P // P - 1 and si == 2),
                        )
                nc.scalar.activation(
                    h1[:, m, n * 512 : (n + 1) * 512],
                    ps[:],
                    af.Sign,
                    bias=c_t[:, m : m + 1],
                    scale=s_t[:, m : m + 1],
                )

        # ---- layers 2 and 3: exact +-1 fp8 DoubleRow matmuls ----
        hin = h1
        for li, wname in ((1, "w2p"), (2, "w3p")):
            s_t, c_t = bn[li]
            hout = hpool.tile([P, M_TILES, B], f8, tag="h")
            for m in range(M_TILES):
                wt = wpool.tile([P, M_TILES, P], f8, tag="w")
                nc.sync.dma_start(wt[:], t[wname][m])
                for n in range(NB):
                    ps = pspool.tile([P, 512], f32, tag="ps")
                    for k2 in range(M_TILES // 2):
                        nc.tensor.matmul(
                            ps[:],
                            wt[:, 2 * k2 : 2 * k2 + 2, :],
                            hin[:, 2 * k2 : 2 * k2 + 2, n * 512 : (n + 1) * 512],
                            start=(k2 == 0),
                            stop=(k2 == M_TILES // 2 - 1),
                            perf_mode=mybir.MatmulPerfMode.DoubleRow,
                        )
                    nc.scalar.activation(
                        hout[:, m, n * 512 : (n + 1) * 512],
                        ps[:],
                        af.Sign,
                        bias=c_t[:, m : m + 1],
                        scale=s_t[:, m : m + 1],
                    )
            hin = hout
        h3 = hin

        # ---- layer 4 + log_softmax ----
        for b in range(BCH):
            ps4 = ps4pool.tile([P, 10], f32, tag="ps4")
            for k in range(M_TILES):
                hb = l4pool.tile([P, P], bf16, tag="hb")
                nc.vector.tensor_copy(hb[:], h3[:, k, b * P : (b + 1) * P])
                nc.tensor.matmul(
                    ps4[:], hb[:], w4t[:, 0, k, :], start=(k == 0), stop=False
                )
                nc.tensor.matmul(ps4[:], hb[:], w4t[:, 1, k, :], start=False, stop=False)
            nc.tensor.matmul(
                ps4[:], ones_ext[:], w4t[:, 0, M_TILES, :], start=False, stop=False
            )
            nc.tensor.matmul(
                ps4[:], ones_ext[:], w4t[:, 1, M_TILES, :], start=False, stop=True
            )
            mx = small.tile([P, 1], f32, tag="mx")
            nc.vector.reduce_max(mx[:], ps4[:], axis=mybir.AxisListType.X)
            nmx = small.tile([P, 1], f32, tag="nmx")
            nc.vector.tensor_scalar_mul(nmx[:], mx[:], -1.0)
            ex = small.tile([P, 10], f32, tag="ex")
            se = small.tile([P, 1], f32, tag="se")
            nc.scalar.activation(
                ex[:], ps4[:], af.Exp, bias=nmx[:], scale=1.0, accum_out=se[:]
            )
            ls = small.tile([P, 1], f32, tag="ls")
            nc.scalar.activation(ls[:], se[:], af.Ln)
            off = small.tile([P, 1], f32, tag="off")
            nc.vector.tensor_add(off[:], mx[:], ls[:])
            ot = small.tile([P, 10], f32, tag="ot")
            nc.vector.tensor_scalar_sub(ot[:], ps4[:], off[:])
            nc.sync.dma_start(t["out"][b * P : (b + 1) * P, :], ot[:])

    nc.compile()
    return nc


def prepare_in_maps(inputs):
    """Host-side packing: binarize weights, fold BN, split/shard x."""
    x = np.asarray(inputs["x"], np.float32).reshape(-1, K1)

    w1p = _pack_weight(_binarize(np.asarray(inputs["w1"], np.float32)), K1P, _BF16)
    w2p = _pack_weight(_binarize(np.asarray(inputs["w2"], np.float32)), H, _FP8)
    w3p = _pack_weight(_binarize(np.asarray(inputs["w3"], np.float32)), H, _FP8)

    # w4 hi/lo split, transposed to [K, 10], plus bias row -> [128, 2, 49, 10]
    w4 = np.asarray(inputs["w4"], np.float32)
    b4 = np.asarray(inputs["b4"], np.float32)
    w4T = np.ascontiguousarray(w4.T)  # [6144, 10]
    w4hi = w4T.astype(_BF16)
    w4lo = (w4T - w4hi.astype(np.float32)).astype(_BF16)
    b4hi = b4.astype(_BF16)
    b4lo = (b4 - b4hi.astype(np.float32)).astype(_BF16)
    w4hl = np.zeros((2, M_TILES + 1, P, 10), _BF16)
    w4hl[0, :M_TILES] = w4hi.reshape(M_TILES, P, 10)
    w4hl[1, :M_TILES] = w4lo.reshape(M_TILES, P, 10)
    w4hl[0, M_TILES, 0] = b4hi
    w4hl[1, M_TILES, 0] = b4lo
    w4hl = np.ascontiguousarray(w4hl.transpose(2, 0, 1, 3))  # [128, 2, 49, 10]

    sc = {}
    for i in (1, 2, 3):
        g = np.asarray(inputs[f"g{i}"], np.float32)
        be = np.asarray(inputs[f"be{i}"], np.float32)
        m = np.asarray(inputs[f"m{i}"], np.float32)
        v = np.asarray(inputs[f"v{i}"], np.float32)
        b = np.asarray(inputs[f"b{i}"], np.float32)
        s = g / np.sqrt(v + np.float32(EPS))
        c = (b - m) * s + be
        sc[f"s{i}"] = np.ascontiguousarray(s.reshape(M_TILES, P).T)
        sc[f"c{i}"] = np.ascontiguousarray(c.reshape(M_TILES, P).T)

    # x: 3-way bf16 split, then pad + shard + pack per core
    x_hi = x.astype(_BF16)
    r = x - x_hi.astype(np.float32)
    x_mid = r.astype(_BF16)
    x_lo = (r - x_mid.astype(np.float32)).astype(_BF16)

    in_maps = []
    for core in range(N_CORES):
        sl = slice(core * B, (core + 1) * B)
        im = {
            "w1p": w1p,
            "w2p": w2p,
            "w3p": w3p,
            "w4hl": w4hl,
            **sc,
        }
        for nm, arr in (("xhi", x_hi), ("xmid", x_mid), ("xlo", x_lo)):
            xc = np.zeros((B, K1P), _BF16)
            xc[:, :K1] = arr[sl]
            im[nm] = _pack_rhs(xc)
        in_maps.append(im)
    return in_maps


_NC_CACHE = []


def kernel(**inputs):
    from concourse.bass_utils import run_bass_kernel_spmd

    if not _NC_CACHE:
        _NC_CACHE.append(build_nc())
    nc = _NC_CACHE[0]

    in_maps = prepare_in_maps(inputs)
    res = run_bass_kernel_spmd(nc, in_maps, core_ids=list(range(N_CORES)))
    return np.concatenate([r["out"] for r in res.results], axis=0)


# revision 4
# speedup vs baseline: 1.1082x; 1.1082x over previous
"""Trainium2 Bass kernel for the binarized MLP (BNN) problem.

Network (eval mode):
  h1 = sign(bn1(x @ sign(w1).T + b1))        x: [8192, 784]
  h2 = sign(bn2(h1 @ sign(w2).T + b2))       hidden: 6144
  h3 = sign(bn3(h2 @ sign(w3).T + b3))
  out = log_softmax(h3 @ w4.T + b4)          out: [8192, 10]
(clip(-1,1) before sign does not change sign, so it is dropped.)

Strategy:
  * Data-parallel over the batch: 8 cores x 1024 rows, no collectives.
  * All activations live transposed in SBUF as hT[H, B] so each layer's
    output feeds the next layer's matmul rhs directly (zero transposes).
  * BN + bias + clip + binarize folds to sign(h*s + c) with
    s = g*rsqrt(v+eps), c = (b - m)*s + be  -> one scalar-engine
    activation (Sign) per psum tile with per-partition scale/bias.
  * Layer 1 (real-valued x, contraction 784->pad 896): x is split into
    2 fp16 planes (hi/lo) so the fp16 matmuls reproduce fp32 precision
    (residual 2^-23; the PE keeps fp16 denormals, verified on HW);
    weights are exact +-1 in fp16.
  * Layers 2/3 (+-1 x +-1, contraction 6144): fp8e4m3 with DoubleRow
    perf mode - products and fp32 PSUM accumulation are exact.
  * Layer 4: w4 split hi/lo bf16, h3 (+-1, exact) converted fp8->bf16
    per tile; bias b4 enters via an extra contraction row. log_softmax
    on-device (reduce_max, Exp with accumulate, Ln, subtract).
"""

import numpy as np
import ml_dtypes

H = 6144
B_TOTAL = 8192
N_CORES = 8
B = B_TOTAL // N_CORES  # 1024 rows per core
K1 = 784
K1P = 896  # 7 * 128
EPS = 1e-5
P = 128
M_TILES = H // P  # 48
NB = B // 512  # psum-width chunks per core
BCH = B // P  # 8 output row-chunks per core

_BF16 = ml_dtypes.bfloat16
_FP8 = ml_dtypes.float8_e4m3
_FP16 = np.float16


def _binarize(w):
    return np.where(w >= 0, np.float32(1.0), np.float32(-1.0))


def _pack_weight(wb, kpad, dtype):
    """[Hout, K] +-1 matrix -> [Hout/128, 128, kpad/128, 128] tiles where
    pack[m, p, ko, j] = wb[m*128 + j, ko*128 + p] (lhsT layout)."""
    hout, k = wb.shape
    if k < kpad:
        wb = np.concatenate([wb, np.zeros((hout, kpad - k), np.float32)], axis=1)
    return np.ascontiguousarray(
        wb.reshape(hout // P, P, kpad // P, P).transpose(0, 3, 2, 1)
    ).astype(dtype)


def _pack_rhs(xc):
    """[B, K] -> [128, K/128, B] with pack[p, ko, b] = xc[b, ko*128+p]."""
    b, k = xc.shape
    return np.ascontiguousarray(xc.T.reshape(k // P, P, b).transpose(1, 0, 2))


def build_nc():
    """Build the (single-program, run-on-8-cores) Bass kernel."""
    import concourse.bass as bass
    import concourse.tile as tile
    import concourse.mybir as mybir
    from concourse import bacc

    af = mybir.ActivationFunctionType
    f32 = mybir.dt.float32
    bf16 = mybir.dt.bfloat16
    f16 = mybir.dt.float16
    f8 = mybir.dt.float8e4

    nc = bacc.Bacc(
        "TRN2",
        target_bir_lowering=False,
        debug=False,
        enable_asserts=False,
        num_devices=N_CORES,
    )

    t = {}
    for nm in ("xhi", "xlo"):
        t[nm] = nc.dram_tensor(nm, [P, K1P // P, B], f16, kind="ExternalInput").ap()
    t["w1p"] = nc.dram_tensor(
        "w1p", [M_TILES, P, K1P // P, P], f16, kind="ExternalInput"
    ).ap()
    for nm in ("w2p", "w3p"):
        t[nm] = nc.dram_tensor(
            nm, [M_TILES, P, M_TILES, P], f8, kind="ExternalInput"
        ).ap()
    t["w4hl"] = nc.dram_tensor(
        "w4hl", [P, 2, M_TILES + 1, 10], bf16, kind="ExternalInput"
    ).ap()
    for i in (1, 2, 3):
        t[f"s{i}"] = nc.dram_tensor(f"s{i}", [P, M_TILES], f32, kind="ExternalInput").ap()
        t[f"c{i}"] = nc.dram_tensor(f"c{i}", [P, M_TILES], f32, kind="ExternalInput").ap()
    t["out"] = nc.dram_tensor("out", [B, 10], f32, kind="ExternalOutput").ap()

    from contextlib import ExitStack

    with tile.TileContext(nc) as tc, ExitStack() as ctx:
        consts = ctx.enter_context(tc.tile_pool(name="consts", bufs=1))
        xpool = ctx.enter_context(tc.tile_pool(name="x", bufs=1))
        hpool = ctx.enter_context(tc.tile_pool(name="h", bufs=2))
        w1pool = ctx.enter_context(tc.tile_pool(name="w1", bufs=2))
        wpool = ctx.enter_context(tc.tile_pool(name="w", bufs=3))
        pspool = ctx.enter_context(tc.tile_pool(name="ps", bufs=2, space="PSUM"))
        ps4pool = ctx.enter_context(tc.tile_pool(name="ps4", bufs=2, space="PSUM"))
        l4pool = ctx.enter_context(tc.tile_pool(name="l4", bufs=4))
        small = ctx.enter_context(tc.tile_pool(name="small", bufs=4))

        # ---- one-time loads ----
        bn = []
        for i in (1, 2, 3):
            s_t = consts.tile([P, M_TILES], f32, tag=f"s{i}")
            nc.sync.dma_start(s_t[:], t[f"s{i}"][:])
            c_t = consts.tile([P, M_TILES], f32, tag=f"c{i}")
            nc.sync.dma_start(c_t[:], t[f"c{i}"][:])
            bn.append((s_t, c_t))
        w4t = consts.tile([P, 2, M_TILES + 1, 10], bf16, tag="w4")
        nc.sync.dma_start(w4t[:], t["w4hl"][:])
        xs = []
        for nm in ("xhi", "xlo"):
            xt = xpool.tile([P, K1P // P, B], f16, tag=nm)
            nc.sync.dma_start(xt[:], t[nm][:])
            xs.append(xt)
        ones_ext = consts.tile([P, P], bf16, tag="ones")
        nc.vector.memset(ones_ext[:], 0.0)
        nc.vector.memset(ones_ext[0:1, :], 1.0)

        # ---- layer 1: 3-way bf16 split of x, K = 896 ----
        s_t, c_t = bn[0]
        h1 = hpool.tile([P, M_TILES, B], f8, tag="h")
        for m in range(M_TILES):
            wt = w1pool.tile([P, K1P // P, P], f16, tag="w1")
            nc.sync.dma_start(wt[:], t["w1p"][m])
            for n in range(NB):
                ps = pspool.tile([P, 512], f32, tag="ps")
                for k in range(K1P // P):
                    for si, xt in enumerate(xs):
                        nc.tensor.matmul(
                            ps[:],
                            wt[:, k, :],
                            xt[:, k, n * 512 : (n + 1) * 512],
                            start=(k == 0 and si == 0),
                            stop=(k == K1P // P - 1 and si == len(xs) - 1),
                        )
                nc.scalar.activation(
                    h1[:, m, n * 512 : (n + 1) * 512],
                    ps[:],
                    af.Sign,
                    bias=c_t[:, m : m + 1],
                    scale=s_t[:, m : m + 1],
                )

        # ---- layers 2 and 3: exact +-1 fp8 DoubleRow matmuls ----
        hin = h1
        for li, wname in ((1, "w2p"), (2, "w3p")):
            s_t, c_t = bn[li]
            hout = hpool.tile([P, M_TILES, B], f8, tag="h")
            for m in range(M_TILES):
                wt = wpool.tile([P, M_TILES, P], f8, tag="w")
                nc.sync.dma_start(wt[:], t[wname][m])
                for n in range(NB):
                    ps = pspool.tile([P, 512], f32, tag="ps")
                    for k2 in range(M_TILES // 2):
                        nc.tensor.matmul(
                            ps[:],
                            wt[:, 2 * k2 : 2 * k2 + 2, :],
                            hin[:, 2 * k2 : 2 * k2 + 2, n * 512 : (n + 1) * 512],
                            start=(k2 == 0),
                            stop=(k2 == M_TILES // 2 - 1),
                            perf_mode=mybir.MatmulPerfMode.DoubleRow,
                        )
                    nc.scalar.activation(
                        hout[:, m, n * 512 : (n + 1) * 512],
                        ps[:],
                        af.Sign,
                        bias=c_t[:, m : m + 1],
                        scale=s_t[:, m : m + 1],
                    )
            hin = hout
        h3 = hin

        # ---- layer 4 + log_softmax ----
        for b in range(BCH):
            ps4 = ps4pool.tile([P, 10], f32, tag="ps4")
            for k in range(M_TILES):
                hb = l4pool.tile([P, P], bf16, tag="hb")
                nc.vector.tensor_copy(hb[:], h3[:, k, b * P : (b + 1) * P])
                nc.tensor.matmul(
                    ps4[:], hb[:], w4t[:, 0, k, :], start=(k == 0), stop=False
                )
                nc.tensor.matmul(ps4[:], hb[:], w4t[:, 1, k, :], start=False, stop=False)
            nc.tensor.matmul(
                ps4[:], ones_ext[:], w4t[:, 0, M_TILES, :], start=False, stop=False
            )
            nc.tensor.matmul(
                ps4[:], ones_ext[:], w4t[:, 1, M_TILES, :], start=False, stop=True
            )
            mx = small.tile([P, 1], f32, tag="mx")
            nc.vector.reduce_max(mx[:], ps4[:], axis=mybir.AxisListType.X)
            nmx = small.tile([P, 1], f32, tag="nmx")
            nc.vector.tensor_scalar_mul(nmx[:], mx[:], -1.0)
            ex = small.tile([P, 10], f32, tag="ex")
            se = small.tile([P, 1], f32, tag="se")
            nc.scalar.activation(
                ex[:], ps4[:], af.Exp, bias=nmx[:], scale=1.0, accum_out=se[:]
            )
            ls = small.tile([P, 1], f32, tag="ls")
            nc.scalar.activation(ls[:], se[:], af.Ln)
            off = small.tile([P, 1], f32, tag="off")
            nc.vector.tensor_add(off[:], mx[:], ls[:])
            ot = small.tile([P, 10], f32, tag="ot")
            nc.vector.tensor_scalar_sub(ot[:], ps4[:], off[:])
            nc.sync.dma_start(t["out"][b * P : (b + 1) * P, :], ot[:])

    nc.compile()
    return nc


def prepare_in_maps(inputs):
    """Host-side packing: binarize weights, fold BN, split/shard x."""
    x = np.asarray(inputs["x"], np.float32).reshape(-1, K1)

    w1p = _pack_weight(_binarize(np.asarray(inputs["w1"], np.float32)), K1P, _FP16)
    w2p = _pack_weight(_binarize(np.asarray(inputs["w2"], np.float32)), H, _FP8)
    w3p = _pack_weight(_binarize(np.asarray(inputs["w3"], np.float32)), H, _FP8)

    # w4 hi/lo split, transposed to [K, 10], plus bias row -> [128, 2, 49, 10]
    w4 = np.asarray(inputs["w4"], np.float32)
    b4 = np.asarray(inputs["b4"], np.float32)
    w4T = np.ascontiguousarray(w4.T)  # [6144, 10]
    w4hi = w4T.astype(_BF16)
    w4lo = (w4T - w4hi.astype(np.float32)).astype(_BF16)
    b4hi = b4.astype(_BF16)
    b4lo = (b4 - b4hi.astype(np.float32)).astype(_BF16)
    w4hl = np.zeros((2, M_TILES + 1, P, 10), _BF16)
    w4hl[0, :M_TILES] = w4hi.reshape(M_TILES, P, 10)
    w4hl[1, :M_TILES] = w4lo.reshape(M_TILES, P, 10)
    w4hl[0, M_TILES, 0] = b4hi
    w4hl[1, M_TILES, 0] = b4lo
    w4hl = np.ascontiguousarray(w4hl.transpose(2, 0, 1, 3))  # [128, 2, 49, 10]

    sc = {}
    for i in (1, 2, 3):
        g = np.asarray(inputs[f"g{i}"], np.float32)
        be = np.asarray(inputs[f"be{i}"], np.float32)
        m = np.asarray(inputs[f"m{i}"], np.float32)
        v = np.asarray(inputs[f"v{i}"], np.float32)
        b = np.asarray(inputs[f"b{i}"], np.float32)
        s = g / np.sqrt(v + np.float32(EPS))
        c = (b - m) * s + be
        sc[f"s{i}"] = np.ascontiguousarray(s.reshape(M_TILES, P).T)
        sc[f"c{i}"] = np.ascontiguousarray(c.reshape(M_TILES, P).T)

    # x: 2-way fp16 split (PE keeps fp16 denormals), pad + shard + pack
    x_hi = x.astype(_FP16)
    x_lo = (x - x_hi.astype(np.float32)).astype(_FP16)

    in_maps = []
    for core in range(N_CORES):
        sl = slice(core * B, (core + 1) * B)
        im = {
            "w1p": w1p,
            "w2p": w2p,
            "w3p": w3p,
            "w4hl": w4hl,
            **sc,
        }
        for nm, arr in (("xhi", x_hi), ("xlo", x_lo)):
            xc = np.zeros((B, K1P), _FP16)
            xc[:, :K1] = arr[sl]
            im[nm] = _pack_rhs(xc)
        in_maps.append(im)
    return in_maps


_NC_CACHE = []


def kernel(**inputs):
    from concourse.bass_utils import run_bass_kernel_spmd

    if not _NC_CACHE:
        _NC_CACHE.append(build_nc())
    nc = _NC_CACHE[0]

    in_maps = prepare_in_maps(inputs)
    res = run_bass_kernel_spmd(nc, in_maps, core_ids=list(range(N_CORES)))
    return np.concatenate([r["out"] for r in res.results], axis=0)


# revision 7
# speedup vs baseline: 1.1343x; 1.0235x over previous
"""Trainium2 Bass kernel for the binarized MLP (BNN) problem.

Network (eval mode):
  h1 = sign(bn1(x @ sign(w1).T + b1))        x: [8192, 784]
  h2 = sign(bn2(h1 @ sign(w2).T + b2))       hidden: 6144
  h3 = sign(bn3(h2 @ sign(w3).T + b3))
  out = log_softmax(h3 @ w4.T + b4)          out: [8192, 10]
(clip(-1,1) before sign does not change sign, so it is dropped.)

Strategy:
  * Data-parallel over the batch: 8 cores x 1024 rows, no collectives.
  * All activations live transposed in SBUF as hT[H, B] so each layer's
    output feeds the next layer's matmul rhs directly (zero transposes).
  * BN + bias + clip + binarize folds to sign(h*s + c) with
    s = g*rsqrt(v+eps), c = (b - m)*s + be  -> one scalar-engine
    activation (Sign) per psum tile with per-partition scale/bias.
  * Layer 1 (real-valued x, contraction 784->pad 896): x is split into
    2 fp16 planes (hi/lo) so the fp16 matmuls reproduce fp32 precision
    (residual 2^-23; the PE keeps fp16 denormals, verified on HW);
    weights are exact +-1 in fp16.
  * Layers 2/3 (+-1 x +-1, contraction 6144): fp8e4m3 with DoubleRow
    perf mode - products and fp32 PSUM accumulation are exact.
  * Layer 4: w4 (split hi/lo bf16, stacked to 20 cols) is the stationary
    operand; h3 streams as fp8 rhs (mixed-dtype matmul, exact) giving
    logitsT [20, B]; hi+lo rows summed + exact fp32 b4 on DVE, then PE
    transposes [10,128] blocks back and log_softmax runs per [128, 10]
    tile (reduce_max, Exp with accumulate, Ln, subtract).
"""

import numpy as np
import ml_dtypes

H = 6144
B_TOTAL = 8192
N_CORES = 8
B = B_TOTAL // N_CORES  # 1024 rows per core
K1 = 784
K1P = 896  # 7 * 128
EPS = 1e-5
P = 128
M_TILES = H // P  # 48
NB = B // 512  # psum-width chunks per core
BCH = B // P  # 8 output row-chunks per core

_BF16 = ml_dtypes.bfloat16
_FP8 = ml_dtypes.float8_e4m3
_FP16 = np.float16


def _binarize(w):
    return np.where(w >= 0, np.float32(1.0), np.float32(-1.0))


def _pack_weight(wb, kpad, dtype):
    """[Hout, K] +-1 matrix -> [Hout/128, 128, kpad/128, 128] tiles where
    pack[m, p, ko, j] = wb[m*128 + j, ko*128 + p] (lhsT layout)."""
    hout, k = wb.shape
    if k < kpad:
        wb = np.concatenate([wb, np.zeros((hout, kpad - k), np.float32)], axis=1)
    return np.ascontiguousarray(
        wb.reshape(hout // P, P, kpad // P, P).transpose(0, 3, 2, 1)
    ).astype(dtype)


def _pack_rhs(xc):
    """[B, K] -> [128, K/128, B] with pack[p, ko, b] = xc[b, ko*128+p]."""
    b, k = xc.shape
    return np.ascontiguousarray(xc.T.reshape(k // P, P, b).transpose(1, 0, 2))


def build_nc():
    """Build the (single-program, run-on-8-cores) Bass kernel."""
    import concourse.bass as bass
    import concourse.tile as tile
    import concourse.mybir as mybir
    from concourse import bacc
    from concourse.masks import make_identity

    af = mybir.ActivationFunctionType
    f32 = mybir.dt.float32
    bf16 = mybir.dt.bfloat16
    f16 = mybir.dt.float16
    f8 = mybir.dt.float8e4

    nc = bacc.Bacc(
        "TRN2",
        target_bir_lowering=False,
        debug=False,
        enable_asserts=False,
        num_devices=N_CORES,
    )

    t = {}
    for nm in ("xhi", "xlo"):
        t[nm] = nc.dram_tensor(nm, [P, K1P // P, B], f16, kind="ExternalInput").ap()
    t["w1p"] = nc.dram_tensor(
        "w1p", [M_TILES, P, K1P // P, P], f16, kind="ExternalInput"
    ).ap()
    for nm in ("w2p", "w3p"):
        t[nm] = nc.dram_tensor(
            nm, [M_TILES, P, M_TILES, P], f8, kind="ExternalInput"
        ).ap()
    t["w4s"] = nc.dram_tensor(
        "w4s", [M_TILES, P, 42], bf16, kind="ExternalInput"
    ).ap()
    t["b4t"] = nc.dram_tensor("b4t", [10, 1], f32, kind="ExternalInput").ap()
    for i in (1, 2, 3):
        t[f"s{i}"] = nc.dram_tensor(f"s{i}", [P, M_TILES], f32, kind="ExternalInput").ap()
        t[f"c{i}"] = nc.dram_tensor(f"c{i}", [P, M_TILES], f32, kind="ExternalInput").ap()
    t["out"] = nc.dram_tensor("out", [B, 10], f32, kind="ExternalOutput").ap()

    from contextlib import ExitStack

    with tile.TileContext(nc) as tc, ExitStack() as ctx:
        consts = ctx.enter_context(tc.tile_pool(name="consts", bufs=1))
        xpool = ctx.enter_context(tc.tile_pool(name="x", bufs=1))
        hpool = ctx.enter_context(tc.tile_pool(name="h", bufs=2))
        w1pool = ctx.enter_context(tc.tile_pool(name="w1", bufs=2))
        wpool = ctx.enter_context(tc.tile_pool(name="w", bufs=3))
        pspool = ctx.enter_context(tc.tile_pool(name="ps", bufs=2, space="PSUM"))
        ps4pool = ctx.enter_context(tc.tile_pool(name="ps4", bufs=2, space="PSUM"))
        ps20pool = ctx.enter_context(tc.tile_pool(name="ps20", bufs=2, space="PSUM"))
        small = ctx.enter_context(tc.tile_pool(name="small", bufs=4))

        # ---- one-time loads ----
        bn = []
        for i in (1, 2, 3):
            s_t = consts.tile([P, M_TILES], f32, tag=f"s{i}")
            nc.sync.dma_start(s_t[:], t[f"s{i}"][:])
            c_t = consts.tile([P, M_TILES], f32, tag=f"c{i}")
            nc.sync.dma_start(c_t[:], t[f"c{i}"][:])
            bn.append((s_t, c_t))
        w4sb = consts.tile([P, M_TILES, 42], bf16, tag="w4")
        nc.sync.dma_start(w4sb[:], t["w4s"].rearrange("k p n -> p k n"))
        b4sb = consts.tile([10, 1], f32, tag="b4")
        nc.sync.dma_start(b4sb[:], t["b4t"][:])
        ident10 = consts.tile([10, 10], f32, tag="ident")
        make_identity(nc, ident10[:])
        xs = []
        for nm in ("xhi", "xlo"):
            xt = xpool.tile([P, K1P // P, B], f16, tag=nm)
            for k in range(K1P // P):
                nc.sync.dma_start(xt[:, k, :], t[nm][:, k, :])
            xs.append(xt)

        # ---- layer 1: 3-way bf16 split of x, K = 896 ----
        s_t, c_t = bn[0]
        h1 = hpool.tile([P, M_TILES, B], f8, tag="h")
        for m in range(M_TILES):
            wt = w1pool.tile([P, K1P // P, P], f16, tag="w1")
            nc.sync.dma_start(wt[:], t["w1p"][m])
            for n in range(NB):
                ps = pspool.tile([P, 512], f32, tag="ps")
                for k in range(K1P // P):
                    for si, xt in enumerate(xs):
                        nc.tensor.matmul(
                            ps[:],
                            wt[:, k, :],
                            xt[:, k, n * 512 : (n + 1) * 512],
                            start=(k == 0 and si == 0),
                            stop=(k == K1P // P - 1 and si == len(xs) - 1),
                        )
                nc.scalar.activation(
                    h1[:, m, n * 512 : (n + 1) * 512],
                    ps[:],
                    af.Sign,
                    bias=c_t[:, m : m + 1],
                    scale=s_t[:, m : m + 1],
                )

        # ---- layers 2 and 3: exact +-1 fp8 DoubleRow matmuls ----
        hin = h1
        for li, wname in ((1, "w2p"), (2, "w3p")):
            s_t, c_t = bn[li]
            hout = hpool.tile([P, M_TILES, B], f8, tag="h")
            for m in range(M_TILES):
                wt = wpool.tile([P, M_TILES, P], f8, tag="w")
                nc.sync.dma_start(wt[:], t[wname][m])
                for n in range(NB):
                    ps = pspool.tile([P, 512], f32, tag="ps")
                    for k2 in range(M_TILES // 2):
                        nc.tensor.matmul(
                            ps[:],
                            wt[:, 2 * k2 : 2 * k2 + 2, :],
                            hin[:, 2 * k2 : 2 * k2 + 2, n * 512 : (n + 1) * 512],
                            start=(k2 == 0),
                            stop=(k2 == M_TILES // 2 - 1),
                            perf_mode=mybir.MatmulPerfMode.DoubleRow,
                        )
                    nc.scalar.activation(
                        hout[:, m, n * 512 : (n + 1) * 512],
                        ps[:],
                        af.Sign,
                        bias=c_t[:, m : m + 1],
                        scale=s_t[:, m : m + 1],
                    )
            hin = hout
        h3 = hin

        # ---- layer 4 + log_softmax ----
        for n in range(NB):
            ps20 = ps20pool.tile([64, 512], f32, tag="ps20")
            for k in range(M_TILES):
                nc.tensor.matmul(
                    ps20[0:42, :],
                    w4sb[:, k, :],
                    h3[:, k, n * 512 : (n + 1) * 512],
                    start=(k == 0),
                    stop=(k == M_TILES - 1),
                )
            lg = small.tile([10, 512], f32, tag="lg")
            nc.scalar.activation(lg[:], ps20[0:10, :], af.Identity, bias=b4sb[:])
            lgb = small.tile([10, 512], f32, tag="lgb")
            nc.vector.tensor_add(lgb[:], lg[:], ps20[32:42, :])
            for bi in range(4):
                pst = ps4pool.tile([P, 10], f32, tag="pst")
                nc.tensor.transpose(pst[:], lgb[:, bi * P : (bi + 1) * P], ident10[:])
                mx = small.tile([P, 1], f32, tag="mx")
                nc.vector.reduce_max(mx[:], pst[:], axis=mybir.AxisListType.X)
                nmx = small.tile([P, 1], f32, tag="nmx")
                nc.vector.tensor_scalar_mul(nmx[:], mx[:], -1.0)
                ex = small.tile([P, 10], f32, tag="ex")
                se = small.tile([P, 1], f32, tag="se")
                nc.scalar.activation(
                    ex[:], pst[:], af.Exp, bias=nmx[:], scale=1.0, accum_out=se[:]
                )
                ls = small.tile([P, 1], f32, tag="ls")
                nc.scalar.activation(ls[:], se[:], af.Ln)
                off = small.tile([P, 1], f32, tag="off")
                nc.vector.tensor_add(off[:], mx[:], ls[:])
                ot = small.tile([P, 10], f32, tag="ot")
                nc.vector.tensor_scalar_sub(ot[:], pst[:], off[:])
                row = n * 512 + bi * P
                nc.sync.dma_start(t["out"][row : row + P, :], ot[:])

    nc.compile()
    return nc


def prepare_in_maps(inputs):
    """Host-side packing: binarize weights, fold BN, split/shard x."""
    x = np.asarray(inputs["x"], np.float32).reshape(-1, K1)

    w1p = _pack_weight(_binarize(np.asarray(inputs["w1"], np.float32)), K1P, _FP16)
    w2p = _pack_weight(_binarize(np.asarray(inputs["w2"], np.float32)), H, _FP8)
    w3p = _pack_weight(_binarize(np.asarray(inputs["w3"], np.float32)), H, _FP8)

    # w4 hi/lo split, transposed to [K, 10], stacked hi|lo -> [48, 128, 20]
    w4 = np.asarray(inputs["w4"], np.float32)
    b4 = np.asarray(inputs["b4"], np.float32)
    w4T = np.ascontiguousarray(w4.T)  # [6144, 10]
    w4hi = w4T.astype(_BF16)
    w4lo = (w4T - w4hi.astype(np.float32)).astype(_BF16)
    w4s = np.zeros((M_TILES, P, 42), _BF16)
    w4s[:, :, 0:10] = w4hi.reshape(M_TILES, P, 10)
    w4s[:, :, 32:42] = w4lo.reshape(M_TILES, P, 10)
    b4t = np.ascontiguousarray(b4.reshape(10, 1))

    sc = {}
    for i in (1, 2, 3):
        g = np.asarray(inputs[f"g{i}"], np.float32)
        be = np.asarray(inputs[f"be{i}"], np.float32)
        m = np.asarray(inputs[f"m{i}"], np.float32)
        v = np.asarray(inputs[f"v{i}"], np.float32)
        b = np.asarray(inputs[f"b{i}"], np.float32)
        s = g / np.sqrt(v + np.float32(EPS))
        c = (b - m) * s + be
        sc[f"s{i}"] = np.ascontiguousarray(s.reshape(M_TILES, P).T)
        sc[f"c{i}"] = np.ascontiguousarray(c.reshape(M_TILES, P).T)

    # x: 2-way fp16 split (PE keeps fp16 denormals), pad + shard + pack
    x_hi = x.astype(_FP16)
    x_lo = (x - x_hi.astype(np.float32)).astype(_FP16)

    in_maps = []
    for core in range(N_CORES):
        sl = slice(core * B, (core + 1) * B)
        im = {
            "w1p": w1p,
            "w2p": w2p,
            "w3p": w3p,
            "w4s": w4s,
            "b4t": b4t,
            **sc,
        }
        for nm, arr in (("xhi", x_hi), ("xlo", x_lo)):
            xc = np.zeros((B, K1P), _FP16)
            xc[:, :K1] = arr[sl]
            im[nm] = _pack_rhs(xc)
        in_maps.append(im)
    return in_maps


_NC_CACHE = []


def kernel(**inputs):
    from concourse.bass_utils import run_bass_kernel_spmd

    if not _NC_CACHE:
        _NC_CACHE.append(build_nc())
    nc = _NC_CACHE[0]

    in_maps = prepare_in_maps(inputs)
    res = run_bass_kernel_spmd(nc, in_maps, core_ids=list(range(N_CORES)))
    return np.concatenate([r["out"] for r in res.results], axis=0)


# revision 8
# speedup vs baseline: 1.1405x; 1.0054x over previous
"""Trainium2 Bass kernel for the binarized MLP (BNN) problem.

Network (eval mode):
  h1 = sign(bn1(x @ sign(w1).T + b1))        x: [8192, 784]
  h2 = sign(bn2(h1 @ sign(w2).T + b2))       hidden: 6144
  h3 = sign(bn3(h2 @ sign(w3).T + b3))
  out = log_softmax(h3 @ w4.T + b4)          out: [8192, 10]
(clip(-1,1) before sign does not change sign, so it is dropped.)

Strategy:
  * Data-parallel over the batch: 8 cores x 1024 rows, no collectives.
  * All activations live transposed in SBUF as hT[H, B] so each layer's
    output feeds the next layer's matmul rhs directly (zero transposes).
  * BN + bias + clip + binarize folds to sign(h*s + c) with
    s = g*rsqrt(v+eps), c = (b - m)*s + be  -> one scalar-engine
    activation (Sign) per psum tile with per-partition scale/bias.
  * Layer 1 (real-valued x, contraction 784->pad 896): x is split into
    2 fp16 planes (hi/lo) so the fp16 matmuls reproduce fp32 precision
    (residual 2^-23; the PE keeps fp16 denormals, verified on HW);
    weights are exact +-1 in fp16.
  * Layers 2/3 (+-1 x +-1, contraction 6144): fp8e4m3 with DoubleRow
    perf mode - products and fp32 PSUM accumulation are exact.
  * Layer 4: w4 (split hi/lo bf16, stacked to 20 cols) is the stationary
    operand; h3 streams as fp8 rhs (mixed-dtype matmul, exact) giving
    logitsT [20, B]; hi+lo rows summed + exact fp32 b4 on DVE, then PE
    transposes [10,128] blocks back and log_softmax runs per [128, 10]
    tile (reduce_max, Exp with accumulate, Ln, subtract).
"""

import numpy as np
import ml_dtypes

H = 6144
B_TOTAL = 8192
N_CORES = 8
B = B_TOTAL // N_CORES  # 1024 rows per core
K1 = 784
K1P = 896  # 7 * 128
EPS = 1e-5
P = 128
M_TILES = H // P  # 48
NB = B // 512  # psum-width chunks per core
BCH = B // P  # 8 output row-chunks per core

_BF16 = ml_dtypes.bfloat16
_FP8 = ml_dtypes.float8_e4m3
_FP16 = np.float16


def _binarize(w):
    return np.where(w >= 0, np.float32(1.0), np.float32(-1.0))


def _pack_weight(wb, kpad, dtype):
    """[Hout, K] +-1 matrix -> [Hout/128, 128, kpad/128, 128] tiles where
    pack[m, p, ko, j] = wb[m*128 + j, ko*128 + p] (lhsT layout)."""
    hout, k = wb.shape
    if k < kpad:
        wb = np.concatenate([wb, np.zeros((hout, kpad - k), np.float32)], axis=1)
    return np.ascontiguousarray(
        wb.reshape(hout // P, P, kpad // P, P).transpose(0, 3, 2, 1)
    ).astype(dtype)


def _pack_rhs(xc):
    """[B, K] -> [128, K/128, B] with pack[p, ko, b] = xc[b, ko*128+p]."""
    b, k = xc.shape
    return np.ascontiguousarray(xc.T.reshape(k // P, P, b).transpose(1, 0, 2))


def build_nc():
    """Build the (single-program, run-on-8-cores) Bass kernel."""
    import concourse.bass as bass
    import concourse.tile as tile
    import concourse.mybir as mybir
    from concourse import bacc
    from concourse.masks import make_identity

    af = mybir.ActivationFunctionType
    f32 = mybir.dt.float32
    bf16 = mybir.dt.bfloat16
    f16 = mybir.dt.float16
    f8 = mybir.dt.float8e4

    nc = bacc.Bacc(
        "TRN2",
        target_bir_lowering=False,
        debug=False,
        enable_asserts=False,
        num_devices=N_CORES,
    )

    t = {}
    for nm in ("xhi", "xlo"):
        t[nm] = nc.dram_tensor(nm, [P, K1P // P, B], f16, kind="ExternalInput").ap()
    t["w1p"] = nc.dram_tensor(
        "w1p", [M_TILES, P, K1P // P, P], f16, kind="ExternalInput"
    ).ap()
    for nm in ("w2p", "w3p"):
        t[nm] = nc.dram_tensor(
            nm, [M_TILES, P, M_TILES, P], f8, kind="ExternalInput"
        ).ap()
    t["w4s"] = nc.dram_tensor(
        "w4s", [M_TILES, P, 42], bf16, kind="ExternalInput"
    ).ap()
    t["b4t"] = nc.dram_tensor("b4t", [10, 1], f32, kind="ExternalInput").ap()
    for i in (1, 2, 3):
        t[f"s{i}"] = nc.dram_tensor(f"s{i}", [P, M_TILES], f32, kind="ExternalInput").ap()
        t[f"c{i}"] = nc.dram_tensor(f"c{i}", [P, M_TILES], f32, kind="ExternalInput").ap()
    t["out"] = nc.dram_tensor("out", [B, 10], f32, kind="ExternalOutput").ap()

    from contextlib import ExitStack

    with tile.TileContext(nc) as tc, ExitStack() as ctx:
        consts = ctx.enter_context(tc.tile_pool(name="consts", bufs=1))
        xpool = ctx.enter_context(tc.tile_pool(name="x", bufs=1))
        hpool = ctx.enter_context(tc.tile_pool(name="h", bufs=2))
        w1pool = ctx.enter_context(tc.tile_pool(name="w1", bufs=2))
        wpool = ctx.enter_context(tc.tile_pool(name="w", bufs=3))
        pspool = ctx.enter_context(tc.tile_pool(name="ps", bufs=3, space="PSUM"))
        ps4pool = ctx.enter_context(tc.tile_pool(name="ps4", bufs=2, space="PSUM"))
        ps20pool = ctx.enter_context(tc.tile_pool(name="ps20", bufs=2, space="PSUM"))
        small = ctx.enter_context(tc.tile_pool(name="small", bufs=4))

        # ---- one-time loads ----
        xs = []
        for nm in ("xhi", "xlo"):
            xt = xpool.tile([P, K1P // P, B], f16, tag=nm)
            nc.sync.dma_start(xt[:], t[nm][:])
            xs.append(xt)
        bn = []
        for i in (1, 2, 3):
            s_t = consts.tile([P, M_TILES], f32, tag=f"s{i}")
            nc.gpsimd.dma_start(s_t[:], t[f"s{i}"][:])
            c_t = consts.tile([P, M_TILES], f32, tag=f"c{i}")
            nc.gpsimd.dma_start(c_t[:], t[f"c{i}"][:])
            bn.append((s_t, c_t))
        w4sb = consts.tile([P, M_TILES, 42], bf16, tag="w4")
        nc.gpsimd.dma_start(w4sb[:], t["w4s"].rearrange("k p n -> p k n"))
        b4sb = consts.tile([10, 1], f32, tag="b4")
        nc.gpsimd.dma_start(b4sb[:], t["b4t"][:])
        ident10 = consts.tile([10, 10], f32, tag="ident")
        make_identity(nc, ident10[:])

        # ---- layer 1: 3-way bf16 split of x, K = 896 ----
        s_t, c_t = bn[0]
        h1 = hpool.tile([P, M_TILES, B], f8, tag="h")
        for m in range(M_TILES):
            wt = w1pool.tile([P, K1P // P, P], f16, tag="w1")
            nc.sync.dma_start(wt[:], t["w1p"][m])
            for n in range(NB):
                ps = pspool.tile([P, 512], f32, tag="ps")
                for k in range(K1P // P):
                    for si, xt in enumerate(xs):
                        nc.tensor.matmul(
                            ps[:],
                            wt[:, k, :],
                            xt[:, k, n * 512 : (n + 1) * 512],
                            start=(k == 0 and si == 0),
                            stop=(k == K1P // P - 1 and si == len(xs) - 1),
                        )
                nc.scalar.activation(
                    h1[:, m, n * 512 : (n + 1) * 512],
                    ps[:],
                    af.Sign,
                    bias=c_t[:, m : m + 1],
                    scale=s_t[:, m : m + 1],
                )

        # ---- layers 2 and 3: exact +-1 fp8 DoubleRow matmuls ----
        hin = h1
        for li, wname in ((1, "w2p"), (2, "w3p")):
            s_t, c_t = bn[li]
            hout = hpool.tile([P, M_TILES, B], f8, tag="h")
            for m in range(M_TILES):
                wt = wpool.tile([P, M_TILES, P], f8, tag="w")
                nc.sync.dma_start(wt[:], t[wname][m])
                for n in range(NB):
                    ps = pspool.tile([P, 512], f32, tag="ps")
                    for k2 in range(M_TILES // 2):
                        nc.tensor.matmul(
                            ps[:],
                            wt[:, 2 * k2 : 2 * k2 + 2, :],
                            hin[:, 2 * k2 : 2 * k2 + 2, n * 512 : (n + 1) * 512],
                            start=(k2 == 0),
                            stop=(k2 == M_TILES // 2 - 1),
                            perf_mode=mybir.MatmulPerfMode.DoubleRow,
                        )
                    nc.scalar.activation(
                        hout[:, m, n * 512 : (n + 1) * 512],
                        ps[:],
                        af.Sign,
                        bias=c_t[:, m : m + 1],
                        scale=s_t[:, m : m + 1],
                    )
            hin = hout
        h3 = hin

        # ---- layer 4 + log_softmax ----
        softmax_parts = []
        for n in range(NB):
            ps20 = ps20pool.tile([64, 512], f32, tag="ps20")
            for k in range(M_TILES):
                nc.tensor.matmul(
                    ps20[0:42, :],
                    w4sb[:, k, :],
                    h3[:, k, n * 512 : (n + 1) * 512],
                    start=(k == 0),
                    stop=(k == M_TILES - 1),
                )
            lg = small.tile([10, 512], f32, tag="lg")
            nc.vector.tensor_scalar_add(lg[:], ps20[0:10, :], b4sb[:])
            lgb = small.tile([10, 512], f32, tag="lgb")
            nc.vector.tensor_add(lgb[:], lg[:], ps20[32:42, :])
            # pass A per 128-row block: transpose, max, shift, exp(+sum)
            for bi in range(4):
                pst = ps4pool.tile([P, 10], f32, tag="pst")
                nc.tensor.transpose(pst[:], lgb[:, bi * P : (bi + 1) * P], ident10[:])
                mx = small.tile([P, 1], f32, tag="mx")
                nc.vector.reduce_max(mx[:], pst[:], axis=mybir.AxisListType.X)
                otp = small.tile([P, 10], f32, tag="otp", bufs=8)
                nc.vector.tensor_scalar_sub(otp[:], pst[:], mx[:])
                nmx = small.tile([P, 1], f32, tag="nmx")
                nc.vector.tensor_scalar_mul(nmx[:], mx[:], -1.0)
                ex = small.tile([P, 10], f32, tag="ex")
                se = small.tile([P, 1], f32, tag="se", bufs=8)
                nc.scalar.activation(
                    ex[:], pst[:], af.Exp, bias=nmx[:], scale=1.0, accum_out=se[:]
                )
                softmax_parts.append((n * 512 + bi * P, otp, se))
        # pass B: all Ln's grouped (single act-table load), subtract, store
        for row, otp, se in softmax_parts:
            ls = small.tile([P, 1], f32, tag="ls")
            nc.scalar.activation(ls[:], se[:], af.Ln)
            ot = small.tile([P, 10], f32, tag="ot")
            nc.vector.tensor_scalar_sub(ot[:], otp[:], ls[:])
            nc.sync.dma_start(t["out"][row : row + P, :], ot[:])

    nc.compile()
    return nc


def prepare_in_maps(inputs):
    """Host-side packing: binarize weights, fold BN, split/shard x."""
    x = np.asarray(inputs["x"], np.float32).reshape(-1, K1)

    w1p = _pack_weight(_binarize(np.asarray(inputs["w1"], np.float32)), K1P, _FP16)
    w2p = _pack_weight(_binarize(np.asarray(inputs["w2"], np.float32)), H, _FP8)
    w3p = _pack_weight(_binarize(np.asarray(inputs["w3"], np.float32)), H, _FP8)

    # w4 hi/lo split, transposed to [K, 10], stacked hi|lo -> [48, 128, 20]
    w4 = np.asarray(inputs["w4"], np.float32)
    b4 = np.asarray(inputs["b4"], np.float32)
    w4T = np.ascontiguousarray(w4.T)  # [6144, 10]
    w4hi = w4T.astype(_BF16)
    w4lo = (w4T - w4hi.astype(np.float32)).astype(_BF16)
    w4s = np.zeros((M_TILES, P, 42), _BF16)
    w4s[:, :, 0:10] = w4hi.reshape(M_TILES, P, 10)
    w4s[:, :, 32:42] = w4lo.reshape(M_TILES, P, 10)
    b4t = np.ascontiguousarray(b4.reshape(10, 1))

    sc = {}
    for i in (1, 2, 3):
        g = np.asarray(inputs[f"g{i}"], np.float32)
        be = np.asarray(inputs[f"be{i}"], np.float32)
        m = np.asarray(inputs[f"m{i}"], np.float32)
        v = np.asarray(inputs[f"v{i}"], np.float32)
        b = np.asarray(inputs[f"b{i}"], np.float32)
        s = g / np.sqrt(v + np.float32(EPS))
        c = (b - m) * s + be
        sc[f"s{i}"] = np.ascontiguousarray(s.reshape(M_TILES, P).T)
        sc[f"c{i}"] = np.ascontiguousarray(c.reshape(M_TILES, P).T)

    # x: 2-way fp16 split (PE keeps fp16 denormals), pad + shard + pack
    x_hi = x.astype(_FP16)
    x_lo = (x - x_hi.astype(np.float32)).astype(_FP16)

    in_maps = []
    for core in range(N_CORES):
        sl = slice(core * B, (core + 1) * B)
        im = {
            "w1p": w1p,
            "w2p": w2p,
            "w3p": w3p,
            "w4s": w4s,
            "b4t": b4t,
            **sc,
        }
        for nm, arr in (("xhi", x_hi), ("xlo", x_lo)):
            xc = np.zeros((B, K1P), _FP16)
            xc[:, :K1] = arr[sl]
            im[nm] = _pack_rhs(xc)
        in_maps.append(im)
    return in_maps


_NC_CACHE = []


def kernel(**inputs):
    from concourse.bass_utils import run_bass_kernel_spmd

    if not _NC_CACHE:
        _NC_CACHE.append(build_nc())
    nc = _NC_CACHE[0]

    in_maps = prepare_in_maps(inputs)
    res = run_bass_kernel_spmd(nc, in_maps, core_ids=list(range(N_CORES)))
    return np.concatenate([r["out"] for r in res.results], axis=0)
